# revision 31
# baseline (speedup 1.0000x reference)
"""Trainium2 kernel for nn_DirectForce (gnn_message_passing).

Math (see reference):
    h   = softplus(X @ w1 + b1) - log(2)          per-edge MLP        [E, 64]
    mag = h @ w2 + b2                                                  [E, 1]
    mag = mag - mean_over_center(mag)[center]      scatter-mean debias
    pair-average mag between each directed edge and its reverse edge
    F   = segment_sum(mag * unit_vec, center)                          [N, 3]

The pair keys (center+neigh+length+|unit|) are identical exactly for the two
directions of each undirected edge (reverse edge has negated vector, same
length), so the sorted-pair averaging pairs each edge with its reverse.  Since
unit_rev = -unit, the pair-averaged scatter reduces algebraically to
    F = segsum(0.5*mag*unit, center) - segsum(0.5*mag*unit, neigh)
which removes the argsort entirely (verified to 2.5e-8 vs the reference).

Device (8 NeuronCores, SPMD, edges partitioned contiguously 200k/core):
  - ScalarE is the critical engine: softplus needs Exp then Ln (the pwp
    table builds on this stack have no real Softplus spline -- act2_1p /
    func id 97 is a placeholder, verified wrong on HW; ACT Softplus func id
    misses the CAM), and ACT runs 1 elem/cycle/lane on f32 input (~1.28
    cyc on bf16 input, so intermediates that ACT reads stay f32).
    Floor = 2 passes x 64 h-dims/edge / 128 lanes @ 1.2 GHz ~ 167us + per-op
    overhead.  Everything else is arranged to hide under it:
  - features bf16 (2x less HBM than f32, DMA ~145us < ScalarE ~200us)
  - mm1: per superchunk (1024 edges) two bf16 matmuls with zero-padded
    weights [w1;0], [0;w1] accumulate stacked z = [zA; zB] [128, 512] f32 in
    PSUM; z-groups of 3 superchunks share one 3-bank PSUM tile
  - Exp(z + b1) -> e f32 slices of a wide [128, 6144] panel (read straight
    from PSUM, 66 ops); one wide Ln(e + 1) -> h bf16 per panel (18 ops);
    natural_log_exp table set pinned so the ACT table loads exactly once
  - mm2: per superchunk one matmul, lhsT = w2 block-diag [128, 2] bf16 ->
    [2, 512] f32 at base partitions 0/32/64 of a packed [66, 512] mag bank;
    one DVE copy per 3 superchunks + one plain DMA per [2,512] pair (a
    partition-strided gather DMA drops the odd partitions -- HW-verified).
  - software pipeline: each panel's Ln is emitted 2 z-groups after the
    panel completes and its mm2 batch 4 further groups later, so the Exp
    that follows an Ln reads a z the PE finished before the Ln began and
    retired-batch PE ops never gate an upcoming Exp's mm1 producers
    (removes most of the ~2.5us/super-tile boundary stalls; HAM cold
    restarts after the PE's Ln-window idle cause the residual ~1us).
  - PSUM budget: 2x3 z banks + 2x1 mag banks = 8 (all of PSUM).
Host post (index-structured tail, ~6% of input bytes, numpy):
  - debias via bincount, unit vectors, the two segment sums above.
"""

import os

import numpy as np

N_ATOMS = 50000
E_TOT = 1600000
D_FEAT = 128
H_DIM = 64
N_CORES = 8
EC = E_TOT // N_CORES          # 200000 edges per core
SC = 1024                      # edges per superchunk (2 stacked chunks of 512)
NSC = (EC + SC - 1) // SC      # 196 superchunks
ECP = NSC * SC                 # 200704 padded edges per core
GRP = 3                        # superchunks per z-group (3 PSUM banks)
NGR = (NSC + GRP - 1) // GRP   # 66 groups (65 full + 1 of size 1)

# input-tile taper (units: superchunks); 8-SC (2 MiB bf16) steady state
TILE_SIZES = [1, 1, 2, 4] + [8] * 23 + [2, 1, 1]
assert sum(TILE_SIZES) == NSC
NTILES = len(TILE_SIZES)
XT_MAX = max(TILE_SIZES) * SC  # 8192

# ACT super-tiles (units: z-groups): small head for fast pipeline ramp,
# small tail for a short drain
ZS_SIZES = [1, 1, 2] + [4] * 15 + [1, 1]
assert sum(ZS_SIZES) == NGR
ZS_MAX = max(ZS_SIZES) * GRP * 512  # 6144 columns

# per-superchunk (tile index, superchunk offset within tile)
SC2TILE = []
for _ti, _sz in enumerate(TILE_SIZES):
    for _j in range(_sz):
        SC2TILE.append((_ti, _j))

_CACHE = {}
LAST_RESULTS = None


def _patch_act_tables():
    """Make Exp and Ln resolve to the single table set that contains both
    (natural_log_exp_and_others) so the ACT table is loaded exactly once.
    Table-set ids are positional, so keys/order are preserved."""
    import functools
    import concourse.hw_specs as hw_specs
    import concourse.bacc as bacc_mod
    import concourse.mybir as mybir

    if _CACHE.get("tables_patched"):
        return
    orig = hw_specs.get_activation_tables
    Exp = mybir.ActivationFunctionType.Exp
    Ln = mybir.ActivationFunctionType.Ln

    def patched(arch):
        out = {}
        for name, fns in orig(arch).items():
            if name != "natural_log_exp_and_others":
                fns = fns - {Exp, Ln}
            out[name] = fns
        return out

    cached = functools.cache(patched)
    hw_specs.get_activation_tables = cached
    bacc_mod.get_activation_tables = cached
    _CACHE["tables_patched"] = True


def _build_nc():
    import concourse.bacc as bacc
    import concourse.mybir as mybir
    import concourse.tile as tile

    _patch_act_tables()

    F32 = mybir.dt.float32
    BF16 = mybir.dt.bfloat16
    Exp = mybir.ActivationFunctionType.Exp
    Ln = mybir.ActivationFunctionType.Ln

    nc = bacc.Bacc("TRN2", target_bir_lowering=False, debug=False)
    xt_d = nc.dram_tensor("xt", [NTILES, 128, XT_MAX], BF16, kind="ExternalInput")
    w1a_d = nc.dram_tensor("w1a", [128, 128], BF16, kind="ExternalInput")
    w1b_d = nc.dram_tensor("w1b", [128, 128], BF16, kind="ExternalInput")
    b1_d = nc.dram_tensor("b1s", [128, 1], F32, kind="ExternalInput")
    w2_d = nc.dram_tensor("w2d", [128, 2], BF16, kind="ExternalInput")
    mag_d = nc.dram_tensor("mag", [NGR, 2 * GRP, 512], F32, kind="ExternalOutput")

    with tile.TileContext(nc) as tc:
        with (
            tc.tile_pool(name="wp", bufs=1) as wp,
            tc.tile_pool(name="xp", bufs=4) as xp,
            tc.tile_pool(name="ep", bufs=3) as ep,
            tc.tile_pool(name="hp", bufs=2) as hp,
            tc.tile_pool(name="sp", bufs=3) as sp,
            tc.tile_pool(name="zp", bufs=2, space="PSUM") as zp,
            tc.tile_pool(name="magp", bufs=2, space="PSUM") as magp,
        ):
            w1a = wp.tile([128, 128], BF16, tag="w1a")
            w1b = wp.tile([128, 128], BF16, tag="w1b")
            b1s = wp.tile([128, 1], F32, tag="b1s")
            w2d = wp.tile([128, 2], BF16, tag="w2d")
            # spread the startup weight loads across the three DMA-capable
            # issue queues (gpsimd, sync, scalar)
            nc.gpsimd.dma_start(w1a[:], w1a_d[:])
            nc.scalar.dma_start(w1b[:], w1b_d[:])
            nc.sync.dma_start(b1s[:], b1_d[:])
            nc.gpsimd.dma_start(w2d[:], w2_d[:])

            def emit_mm2_batch(batch):
                # mm2 + mag drain for a retired super-tile; deferred so these
                # PE-queue ops never sit between an Ln and the mm1s the next
                # Exp needs (measured ~2.5us ScalarE stall per boundary)
                for h_sb, gg, gcol, width in batch:
                    gsz = width // 512
                    mag_ps = magp.tile([66, 512], F32, tag="mag")
                    for k in range(gsz):
                        nc.tensor.matmul(
                            mag_ps[32 * k:32 * k + 2, :], w2d[:],
                            h_sb[:, gcol + k * 512:gcol + (k + 1) * 512],
                            start=True, stop=True,
                        )
                    stage = sp.tile([66, 512], F32, tag="stage")
                    nc.vector.tensor_copy(
                        stage[:32 * (gsz - 1) + 2, :],
                        mag_ps[:32 * (gsz - 1) + 2, :],
                    )
                    # NOTE: a single partition-strided gather DMA drops the
                    # second partition of each pair (verified on HW) -- use
                    # one plain DMA per [2, 512] pair, alternating issue
                    # queues so the drains don't serialize on gpsimd
                    for k in range(gsz):
                        eng = nc.gpsimd if k % 2 == 0 else nc.sync
                        eng.dma_start(
                            mag_d[gg, 2 * k:2 * k + 2],
                            stage[32 * k:32 * k + 2, :],
                        )

            # Software pipeline via delayed actions, ticked once per z-group:
            #   - a panel's Ln is emitted 2 z-groups AFTER the panel
            #     completes, so every Exp that follows an Ln on the Scalar
            #     queue reads a z the PE finished well before the Ln began
            #     (kills the ~2.5us ScalarE boundary stall per super-tile)
            #   - the panel's mm2 batch is emitted 4 further z-groups later,
            #     so its PE-queue ops sit behind mm1s that are never on the
            #     Exp critical path and its Ln is long done when PE arrives
            actions = []  # [countdown_in_groups, kind, emit_fn], FIFO

            def tick():
                for a in actions:
                    a[0] -= 1
                while actions and actions[0][0] <= 0:
                    actions.pop(0)[2]()

            def make_ln(e_sb, col, goff):
                def fire():
                    h_sb = hp.tile([128, ZS_MAX], BF16, tag="h")
                    nc.scalar.activation(
                        h_sb[:, :col], e_sb[:, :col], Ln, bias=1.0
                    )
                    batch = [(h_sb, gg, gcol, w) for gg, gcol, w in goff]
                    actions.append([4, "batch", lambda: emit_mm2_batch(batch)])
                return fire

            xt_handles = {}
            g = 0
            for zt, zgroups in enumerate(ZS_SIZES):
                # e stays f32: ACT streams 1.0 cyc/col on f32 input but ~1.28
                # on bf16 input (measured), so bf16 staging loses on ScalarE
                e_sb = ep.tile([128, ZS_MAX], F32, tag="e")
                goff = []   # (group index, col offset, width) within this e
                col = 0
                for _gi in range(zgroups):
                    gsz = min(GRP, NSC - g * GRP)
                    width = gsz * 512
                    if _gi == 0 and zt > 0:
                        # dep-free LDWEIGHTS warm the PE's HAM state after
                        # its Ln-window idle, so the mm1s the next Exps need
                        # don't pay the cold-restart rate (427-540ns vs
                        # 215ns/MM measured)
                        for _ in range(4):
                            nc.tensor.ldweights(w1a[:])
                    z_ps = zp.tile([128, GRP * 512], F32, tag="z")
                    for k in range(gsz):
                        s = g * GRP + k
                        ti, j = SC2TILE[s]
                        if ti not in xt_handles:
                            xt = xp.tile([128, XT_MAX], BF16, tag="xt")
                            tw = TILE_SIZES[ti] * SC
                            nc.sync.dma_start(xt[:, :tw], xt_d[ti, :, :tw])
                            xt_handles[ti] = xt
                        xt = xt_handles[ti]
                        off = j * SC
                        nc.tensor.matmul(
                            z_ps[:, k * 512:(k + 1) * 512], w1a[:],
                            xt[:, off:off + 512], start=True, stop=False,
                        )
                        nc.tensor.matmul(
                            z_ps[:, k * 512:(k + 1) * 512], w1b[:],
                            xt[:, off + 512:off + 1024], start=False, stop=True,
                        )
                    nc.scalar.activation(
                        e_sb[:, col:col + width], z_ps[:, :width], Exp,
                        bias=b1s[:, :1],
                    )
                    goff.append((g, col, width))
                    col += width
                    g += 1
                    tick()
                actions.append([2, "ln", make_ln(e_sb, col, goff)])
            # flush: fire ready batches ahead of the remaining tail Lns so
            # their PE/DVE/DMA work overlaps the final Scalar ops (a batch
            # is always ready -- it only enters the queue when its Ln fires)
            while actions:
                bi = next(
                    (i for i, a in enumerate(actions) if a[1] == "batch"), 0
                )
                actions.pop(bi)[2]()
    nc.compile()
    return nc


def _get_nc():
    if "nc" not in _CACHE:
        _CACHE["nc"] = _build_nc()
    return _CACHE["nc"]


def kernel(features, edge_vectors, edge_lengths, edge_index, w1, b1, w2, b2):
    global LAST_RESULTS
    import ml_dtypes
    from concourse.bass_utils import run_bass_kernel_spmd

    features = np.asarray(features, dtype=np.float32)
    edge_vectors = np.asarray(edge_vectors, dtype=np.float32)
    edge_lengths = np.asarray(edge_lengths, dtype=np.float32)
    edge_index = np.asarray(edge_index)
    w1 = np.asarray(w1, dtype=np.float32)
    b1 = np.asarray(b1, dtype=np.float32).reshape(-1)
    w2 = np.asarray(w2, dtype=np.float32).reshape(-1, 1)
    b2 = np.asarray(b2, dtype=np.float32).reshape(-1)

    bf16 = ml_dtypes.bfloat16

    # replicated small weights, padded for the stacked-z / block-diag tricks
    w1a = np.zeros((128, 128), np.float32)
    w1a[:, :H_DIM] = w1
    w1b = np.zeros((128, 128), np.float32)
    w1b[:, H_DIM:] = w1
    w1a = w1a.astype(bf16)
    w1b = w1b.astype(bf16)
    b1s = np.concatenate([b1, b1]).astype(np.float32).reshape(128, 1)
    w2d = np.zeros((128, 2), np.float32)
    w2d[:H_DIM, 0] = w2[:, 0]
    w2d[H_DIM:, 1] = w2[:, 0]
    w2d = w2d.astype(bf16)

    # shard edges contiguously across cores; per-core bf16 transposed feature
    # panel, pre-tiled so each DMA row is contiguous
    xq_all = features.astype(bf16)
    in_maps = []
    for c in range(N_CORES):
        panel = np.zeros((128, ECP), bf16)
        panel[:, :EC] = xq_all[c * EC:(c + 1) * EC].T
        xt = np.zeros((NTILES, 128, XT_MAX), bf16)
        a = 0
        for ti, size in enumerate(TILE_SIZES):
            w = size * SC
            xt[ti, :, :w] = panel[:, a:a + w]
            a += w
        in_maps.append({"xt": xt, "w1a": w1a, "w1b": w1b, "b1s": b1s, "w2d": w2d})

    nc = _get_nc()
    try:
        res = run_bass_kernel_spmd(nc, in_maps, core_ids=list(range(N_CORES)))
    except Exception:
        # one retry for transient runtime failures
        import time
        time.sleep(2.0)
        res = run_bass_kernel_spmd(nc, in_maps, core_ids=list(range(N_CORES)))
    LAST_RESULTS = res

    # decode mag: [NGR, 6, 512]; flat (g, k, half, col) order is edge order
    mag = np.empty(E_TOT, np.float32)
    for c in range(N_CORES):
        arr = np.asarray(res.results[c]["mag"], np.float32)
        mag[c * EC:(c + 1) * EC] = arr.reshape(-1)[:EC]

    # fold b2 and the shifted-softplus constant: h_ref = h_dev - log(2)
    mag = mag + (b2[0] - np.float32(np.log(2.0)) * w2.sum())

    center = edge_index[0].astype(np.int64)
    neigh = edge_index[1].astype(np.int64)

    # scatter-mean debias per center atom
    cnt = np.bincount(center, minlength=N_ATOMS).astype(np.float32)
    ssum = np.bincount(center, weights=mag.astype(np.float64), minlength=N_ATOMS)
    bias = (ssum / np.maximum(cnt, 1.0)).astype(np.float32)
    mag = mag - bias[center]

    # pair-averaged antisymmetric force assembly (see module docstring)
    unit = edge_vectors / edge_lengths[:, None]
    val = (0.5 * mag)[:, None] * unit  # [E, 3]
    forces = np.zeros((N_ATOMS, 3), np.float32)
    for k in range(3):
        fc = np.bincount(center, weights=val[:, k].astype(np.float64), minlength=N_ATOMS)
        fn = np.bincount(neigh, weights=val[:, k].astype(np.float64), minlength=N_ATOMS)
        forces[:, k] = (fc - fn).astype(np.float32)
    return forces


# revision 33
# speedup vs baseline: 1.0072x; 1.0072x over previous
"""Trainium2 kernel for nn_DirectForce (gnn_message_passing).

Math (see reference):
    h   = softplus(X @ w1 + b1) - log(2)          per-edge MLP        [E, 64]
    mag = h @ w2 + b2                                                  [E, 1]
    mag = mag - mean_over_center(mag)[center]      scatter-mean debias
    pair-average mag between each directed edge and its reverse edge
    F   = segment_sum(mag * unit_vec, center)                          [N, 3]

The pair keys (center+neigh+length+|unit|) are identical exactly for the two
directions of each undirected edge (reverse edge has negated vector, same
length), so the sorted-pair averaging pairs each edge with its reverse.  Since
unit_rev = -unit, the pair-averaged scatter reduces algebraically to
    F = segsum(0.5*mag*unit, center) - segsum(0.5*mag*unit, neigh)
which removes the argsort entirely (verified to 2.5e-8 vs the reference).

Device (8 NeuronCores, SPMD, edges partitioned contiguously 200k/core):
  - ScalarE is the critical engine: softplus needs Exp then Ln (the pwp
    table builds on this stack have no real Softplus spline -- act2_1p /
    func id 97 is a placeholder, verified wrong on HW; ACT Softplus func id
    misses the CAM), and ACT runs 1 elem/cycle/lane on f32 input (~1.28
    cyc on bf16 input, so intermediates that ACT reads stay f32).
    Floor = 2 passes x 64 h-dims/edge / 128 lanes @ 1.2 GHz ~ 167us + per-op
    overhead.  Everything else is arranged to hide under it:
  - features bf16 (2x less HBM than f32, DMA ~145us < ScalarE ~200us)
  - mm1: per superchunk (1024 edges) two bf16 matmuls with zero-padded
    weights [w1;0], [0;w1] accumulate stacked z = [zA; zB] [128, 512] f32 in
    PSUM; z-groups of 3 superchunks share one 3-bank PSUM tile
  - Exp(z + b1) -> e f32 slices of a wide [128, 6144] panel (read straight
    from PSUM, 66 ops); one wide Ln(e + 1) -> h bf16 per panel (18 ops);
    natural_log_exp table set pinned so the ACT table loads exactly once
  - mm2: per superchunk one matmul, lhsT = w2 block-diag [128, 2] bf16 ->
    [2, 512] f32 at base partitions 0/32/64 of a packed [66, 512] mag bank;
    one DVE copy per 3 superchunks + one plain DMA per [2,512] pair (a
    partition-strided gather DMA drops the odd partitions -- HW-verified).
  - software pipeline: each panel's Ln is emitted 2 z-groups after the
    panel completes and its mm2 batch 4 further groups later, so the Exp
    that follows an Ln reads a z the PE finished before the Ln began and
    retired-batch PE ops never gate an upcoming Exp's mm1 producers
    (removes most of the ~2.5us/super-tile boundary stalls; HAM cold
    restarts after the PE's Ln-window idle cause the residual ~1us).
  - PSUM budget: 2x3 z banks + 2x1 mag banks = 8 (all of PSUM).
Host post (index-structured tail, ~6% of input bytes, numpy):
  - debias via bincount, unit vectors, the two segment sums above.
"""

import os

import numpy as np

N_ATOMS = 50000
E_TOT = 1600000
D_FEAT = 128
H_DIM = 64
N_CORES = 8
EC = E_TOT // N_CORES          # 200000 edges per core
SC = 1024                      # edges per superchunk (2 stacked chunks of 512)
NSC = (EC + SC - 1) // SC      # 196 superchunks
ECP = NSC * SC                 # 200704 padded edges per core
GRP = 3                        # superchunks per z-group (3 PSUM banks)
NGR = (NSC + GRP - 1) // GRP   # 66 groups (65 full + 1 of size 1)

# input-tile taper (units: superchunks); 8-SC (2 MiB bf16) steady state
TILE_SIZES = [1, 1, 2, 4] + [8] * 23 + [2, 1, 1]
assert sum(TILE_SIZES) == NSC
NTILES = len(TILE_SIZES)
XT_MAX = max(TILE_SIZES) * SC  # 8192

# ACT super-tiles (units: z-groups): small head for fast pipeline ramp,
# small tail for a short drain
ZS_SIZES = [1, 1, 2] + [4] * 15 + [1, 1]
assert sum(ZS_SIZES) == NGR
ZS_MAX = max(ZS_SIZES) * GRP * 512  # 6144 columns

# per-superchunk (tile index, superchunk offset within tile)
SC2TILE = []
for _ti, _sz in enumerate(TILE_SIZES):
    for _j in range(_sz):
        SC2TILE.append((_ti, _j))

_CACHE = {}
LAST_RESULTS = None


def _patch_act_tables():
    """Make Exp and Ln resolve to the single table set that contains both
    (natural_log_exp_and_others) so the ACT table is loaded exactly once.
    Table-set ids are positional, so keys/order are preserved."""
    import functools
    import concourse.hw_specs as hw_specs
    import concourse.bacc as bacc_mod
    import concourse.mybir as mybir

    if _CACHE.get("tables_patched"):
        return
    orig = hw_specs.get_activation_tables
    Exp = mybir.ActivationFunctionType.Exp
    Ln = mybir.ActivationFunctionType.Ln

    def patched(arch):
        out = {}
        for name, fns in orig(arch).items():
            if name != "natural_log_exp_and_others":
                fns = fns - {Exp, Ln}
            out[name] = fns
        return out

    cached = functools.cache(patched)
    hw_specs.get_activation_tables = cached
    bacc_mod.get_activation_tables = cached
    _CACHE["tables_patched"] = True


def _build_nc():
    import concourse.bacc as bacc
    import concourse.mybir as mybir
    import concourse.tile as tile

    _patch_act_tables()

    F32 = mybir.dt.float32
    BF16 = mybir.dt.bfloat16
    Exp = mybir.ActivationFunctionType.Exp
    Ln = mybir.ActivationFunctionType.Ln

    nc = bacc.Bacc("TRN2", target_bir_lowering=False, debug=False)
    xt_d = nc.dram_tensor("xt", [NTILES, 128, XT_MAX], BF16, kind="ExternalInput")
    w1a_d = nc.dram_tensor("w1a", [128, 128], BF16, kind="ExternalInput")
    w1b_d = nc.dram_tensor("w1b", [128, 128], BF16, kind="ExternalInput")
    b1_d = nc.dram_tensor("b1s", [128, 1], F32, kind="ExternalInput")
    w2_d = nc.dram_tensor("w2d", [128, 2], BF16, kind="ExternalInput")
    mag_d = nc.dram_tensor("mag", [NGR, 2 * GRP, 512], F32, kind="ExternalOutput")

    with tile.TileContext(nc) as tc:
        with (
            tc.tile_pool(name="wp", bufs=1) as wp,
            tc.tile_pool(name="xp", bufs=4) as xp,
            tc.tile_pool(name="ep", bufs=3) as ep,
            tc.tile_pool(name="hp", bufs=2) as hp,
            tc.tile_pool(name="sp", bufs=3) as sp,
            tc.tile_pool(name="zp", bufs=2, space="PSUM") as zp,
            tc.tile_pool(name="magp", bufs=2, space="PSUM") as magp,
        ):
            w1a = wp.tile([128, 128], BF16, tag="w1a")
            w1b = wp.tile([128, 128], BF16, tag="w1b")
            b1s = wp.tile([128, 1], F32, tag="b1s")
            w2d = wp.tile([128, 2], BF16, tag="w2d")
            # spread the startup weight loads across the three DMA-capable
            # issue queues (gpsimd, sync, scalar)
            nc.gpsimd.dma_start(w1a[:], w1a_d[:])
            nc.scalar.dma_start(w1b[:], w1b_d[:])
            nc.sync.dma_start(b1s[:], b1_d[:])
            nc.gpsimd.dma_start(w2d[:], w2_d[:])

            def emit_mm2_batch(batch):
                # mm2 + mag drain for a retired super-tile; deferred so these
                # PE-queue ops never sit between an Ln and the mm1s the next
                # Exp needs (measured ~2.5us ScalarE stall per boundary)
                for h_sb, gg, gcol, width in batch:
                    gsz = width // 512
                    mag_ps = magp.tile([66, 512], F32, tag="mag")
                    for k in range(gsz):
                        nc.tensor.matmul(
                            mag_ps[32 * k:32 * k + 2, :], w2d[:],
                            h_sb[:, gcol + k * 512:gcol + (k + 1) * 512],
                            start=True, stop=True,
                        )
                    stage = sp.tile([66, 512], F32, tag="stage")
                    nc.vector.tensor_copy(
                        stage[:32 * (gsz - 1) + 2, :],
                        mag_ps[:32 * (gsz - 1) + 2, :],
                    )
                    # NOTE: a single partition-strided gather DMA drops the
                    # second partition of each pair (verified on HW) -- use
                    # one plain DMA per [2, 512] pair, alternating issue
                    # queues so the drains don't serialize on gpsimd
                    for k in range(gsz):
                        eng = nc.gpsimd if k % 2 == 0 else nc.sync
                        # single_packet: 4 KiB 2-row transfer fits one DMA
                        # packet, trimming per-DMA fixed overhead
                        eng.dma_start(
                            mag_d[gg, 2 * k:2 * k + 2],
                            stage[32 * k:32 * k + 2, :],
                            single_packet=True,
                        )

            # Software pipeline via delayed actions, ticked once per z-group:
            #   - a panel's Ln is emitted 2 z-groups AFTER the panel
            #     completes, so every Exp that follows an Ln on the Scalar
            #     queue reads a z the PE finished well before the Ln began
            #     (kills the ~2.5us ScalarE boundary stall per super-tile)
            #   - the panel's mm2 batch is emitted 4 further z-groups later,
            #     so its PE-queue ops sit behind mm1s that are never on the
            #     Exp critical path and its Ln is long done when PE arrives
            actions = []  # [countdown_in_groups, kind, emit_fn], FIFO

            def tick():
                for a in actions:
                    a[0] -= 1
                while actions and actions[0][0] <= 0:
                    actions.pop(0)[2]()

            def make_ln(e_sb, col, goff):
                def fire():
                    h_sb = hp.tile([128, ZS_MAX], BF16, tag="h")
                    nc.scalar.activation(
                        h_sb[:, :col], e_sb[:, :col], Ln, bias=1.0
                    )
                    batch = [(h_sb, gg, gcol, w) for gg, gcol, w in goff]
                    actions.append([4, "batch", lambda: emit_mm2_batch(batch)])
                return fire

            xt_handles = {}
            g = 0
            for zt, zgroups in enumerate(ZS_SIZES):
                # e stays f32: ACT streams 1.0 cyc/col on f32 input but ~1.28
                # on bf16 input (measured), so bf16 staging loses on ScalarE
                e_sb = ep.tile([128, ZS_MAX], F32, tag="e")
                goff = []   # (group index, col offset, width) within this e
                col = 0
                for _gi in range(zgroups):
                    gsz = min(GRP, NSC - g * GRP)
                    width = gsz * 512
                    z_ps = zp.tile([128, GRP * 512], F32, tag="z")
                    for k in range(gsz):
                        s = g * GRP + k
                        ti, j = SC2TILE[s]
                        if ti not in xt_handles:
                            xt = xp.tile([128, XT_MAX], BF16, tag="xt")
                            tw = TILE_SIZES[ti] * SC
                            nc.sync.dma_start(xt[:, :tw], xt_d[ti, :, :tw])
                            xt_handles[ti] = xt
                        xt = xt_handles[ti]
                        off = j * SC
                        nc.tensor.matmul(
                            z_ps[:, k * 512:(k + 1) * 512], w1a[:],
                            xt[:, off:off + 512], start=True, stop=False,
                        )
                        nc.tensor.matmul(
                            z_ps[:, k * 512:(k + 1) * 512], w1b[:],
                            xt[:, off + 512:off + 1024], start=False, stop=True,
                        )
                    nc.scalar.activation(
                        e_sb[:, col:col + width], z_ps[:, :width], Exp,
                        bias=b1s[:, :1],
                    )
                    goff.append((g, col, width))
                    col += width
                    g += 1
                    tick()
                actions.append([2, "ln", make_ln(e_sb, col, goff)])
            # flush: fire ready batches ahead of the remaining tail Lns so
            # their PE/DVE/DMA work overlaps the final Scalar ops (a batch
            # is always ready -- it only enters the queue when its Ln fires)
            while actions:
                bi = next(
                    (i for i, a in enumerate(actions) if a[1] == "batch"), 0
                )
                actions.pop(bi)[2]()
    nc.compile()
    return nc


def _get_nc():
    if "nc" not in _CACHE:
        _CACHE["nc"] = _build_nc()
    return _CACHE["nc"]


def kernel(features, edge_vectors, edge_lengths, edge_index, w1, b1, w2, b2):
    global LAST_RESULTS
    import ml_dtypes
    from concourse.bass_utils import run_bass_kernel_spmd

    features = np.asarray(features, dtype=np.float32)
    edge_vectors = np.asarray(edge_vectors, dtype=np.float32)
    edge_lengths = np.asarray(edge_lengths, dtype=np.float32)
    edge_index = np.asarray(edge_index)
    w1 = np.asarray(w1, dtype=np.float32)
    b1 = np.asarray(b1, dtype=np.float32).reshape(-1)
    w2 = np.asarray(w2, dtype=np.float32).reshape(-1, 1)
    b2 = np.asarray(b2, dtype=np.float32).reshape(-1)

    bf16 = ml_dtypes.bfloat16

    # replicated small weights, padded for the stacked-z / block-diag tricks
    w1a = np.zeros((128, 128), np.float32)
    w1a[:, :H_DIM] = w1
    w1b = np.zeros((128, 128), np.float32)
    w1b[:, H_DIM:] = w1
    w1a = w1a.astype(bf16)
    w1b = w1b.astype(bf16)
    b1s = np.concatenate([b1, b1]).astype(np.float32).reshape(128, 1)
    w2d = np.zeros((128, 2), np.float32)
    w2d[:H_DIM, 0] = w2[:, 0]
    w2d[H_DIM:, 1] = w2[:, 0]
    w2d = w2d.astype(bf16)

    # shard edges contiguously across cores; per-core bf16 transposed feature
    # panel, pre-tiled so each DMA row is contiguous
    xq_all = features.astype(bf16)
    in_maps = []
    for c in range(N_CORES):
        panel = np.zeros((128, ECP), bf16)
        panel[:, :EC] = xq_all[c * EC:(c + 1) * EC].T
        xt = np.zeros((NTILES, 128, XT_MAX), bf16)
        a = 0
        for ti, size in enumerate(TILE_SIZES):
            w = size * SC
            xt[ti, :, :w] = panel[:, a:a + w]
            a += w
        in_maps.append({"xt": xt, "w1a": w1a, "w1b": w1b, "b1s": b1s, "w2d": w2d})

    nc = _get_nc()
    try:
        res = run_bass_kernel_spmd(nc, in_maps, core_ids=list(range(N_CORES)))
    except Exception:
        # one retry for transient runtime failures
        import time
        time.sleep(2.0)
        res = run_bass_kernel_spmd(nc, in_maps, core_ids=list(range(N_CORES)))
    LAST_RESULTS = res

    # decode mag: [NGR, 6, 512]; flat (g, k, half, col) order is edge order
    mag = np.empty(E_TOT, np.float32)
    for c in range(N_CORES):
        arr = np.asarray(res.results[c]["mag"], np.float32)
        mag[c * EC:(c + 1) * EC] = arr.reshape(-1)[:EC]

    # fold b2 and the shifted-softplus constant: h_ref = h_dev - log(2)
    mag = mag + (b2[0] - np.float32(np.log(2.0)) * w2.sum())

    center = edge_index[0].astype(np.int64)
    neigh = edge_index[1].astype(np.int64)

    # scatter-mean debias per center atom
    cnt = np.bincount(center, minlength=N_ATOMS).astype(np.float32)
    ssum = np.bincount(center, weights=mag.astype(np.float64), minlength=N_ATOMS)
    bias = (ssum / np.maximum(cnt, 1.0)).astype(np.float32)
    mag = mag - bias[center]

    # pair-averaged antisymmetric force assembly (see module docstring)
    unit = edge_vectors / edge_lengths[:, None]
    val = (0.5 * mag)[:, None] * unit  # [E, 3]
    forces = np.zeros((N_ATOMS, 3), np.float32)
    for k in range(3):
        fc = np.bincount(center, weights=val[:, k].astype(np.float64), minlength=N_ATOMS)
        fn = np.bincount(neigh, weights=val[:, k].astype(np.float64), minlength=N_ATOMS)
        forces[:, k] = (fc - fn).astype(np.float32)
    return forces
